# revision 8
# baseline (speedup 1.0000x reference)
"""Causal self-attention (B=4, T=2048, C=1024, H=16) on 8 Trainium2 cores.

Sharding: batch x head-half. Core c handles batch b=c//2 and heads
[8*(c%2), 8*(c%2)+8). Each core computes qkv for its head slice, causal
flash-style attention fully in SBUF, and a partial c_proj; a pairwise
ReduceScatter (cores 2b, 2b+1) sums the two head-halves and leaves each
core with 1024 rows of the final output.

Layouts (per core):
  xT   [1024 C, 2048 tok]   host-pre-transposed, rounded to f32r on DVE
  qT,kT [512 cols, 2048]    f32r, computed as W^T-stationary matmuls
  v_aug [tok, 8 heads x 65] bf16 (65th col = 1.0 -> softmax denominator)
  S^T  [128 ktok, 512 q]    psum; exp on ACT -> attT bf16 (causal mask on
                            diagonal tiles via precomputed bf16 masks)
  PV   attT^T @ v_aug -> [128 q, 65] psum accumulated over ktiles; col 64
       is the softmax denominator; normalize with DVE reciprocal
  y -> yT via PE transpose; proj = yT^T-stationary matmuls + bias
"""
import os
import time
from contextlib import ExitStack

import numpy as np
import ml_dtypes

import concourse.bass as bass
import concourse.mybir as mybir
import concourse.tile as tile
from concourse.masks import make_identity

B, T, C = 4, 2048, 1024
H, HD = 16, 64
NCORES = 8
P = 128
KC = C // P  # 8 contraction chunks
HPC = H // 2  # heads per core
HCOLS = HPC * HD  # 512 qkv columns per core
TOKTILES = T // P  # 16
F32 = mybir.dt.float32
F32R = mybir.dt.float32r
BF16 = mybir.dt.bfloat16


def legalize_waits(nc):
    """This walrus build rejects >1 sem wait per instruction (>2 for
    EventSemaphore): split extras onto preceding same-engine NOPs."""
    for f in nc.m.functions:
        for bb in f.blocks:
            new_insts = []
            for inst in bb.instructions:
                si = inst.sync_info
                cap = 2 if isinstance(inst, mybir.InstEventSemaphore) else 1
                if si is not None and si.on_wait and len(si.on_wait) > cap:
                    waits = list(si.on_wait)
                    extra, keep = waits[:-cap], waits[-cap:]
                    for k, w in enumerate(extra):
                        new_insts.append(
                            mybir.InstNoOp(
                                name=f"{inst.name}-splitw{k}",
                                engine=inst.engine,
                                sync_info=mybir.SyncInfo(on_wait=[w], on_update=[]),
                            )
                        )
                    si.on_wait = keep
                    inst.sync_info = si
                new_insts.append(inst)
            bb.instructions = new_insts


def build_nc(reps: int = 1):
    nc = bass.Bass()
    xt_in = nc.declare_dram_parameter("xt", [C, T], F32, isOutput=False)
    w3_in = nc.declare_dram_parameter("w3", [C, 3 * HCOLS], F32, isOutput=False)
    wp_in = nc.declare_dram_parameter("wp", [HCOLS, C], F32, isOutput=False)
    bq_in = nc.declare_dram_parameter("bq", [4, P, 1], F32, isOutput=False)
    bk_in = nc.declare_dram_parameter("bk", [4, P, 1], F32, isOutput=False)
    bvb_in = nc.declare_dram_parameter("bvb", [P, HCOLS], F32, isOutput=False)
    bpb_in = nc.declare_dram_parameter("bpb", [P, C], F32, isOutput=False)
    masks_in = nc.declare_dram_parameter("masks", [4, P, 512], BF16, isOutput=False)
    out_p = nc.declare_dram_parameter("out_part", [T // 2, C], F32, isOutput=True)

    with tile.TileContext(nc) as tc, ExitStack() as top:
        dram = top.enter_context(tc.tile_pool(name="dram", bufs=1, space="DRAM"))
        partial = [dram.tile([512, C], F32, tag=f"partial{g}", name=f"partial{g}") for g in range(4)]
        rs_out = [dram.tile([256, C], F32, tag=f"rs{g}", name=f"rs{g}") for g in range(4)]

        const = top.enter_context(tc.tile_pool(name="const", bufs=1))
        masks = [const.tile([P, 512], BF16, tag=f"mask{d}", name=f"mask{d}") for d in range(4)]
        for d in range(4):
            nc.sync.dma_start(masks[d][:], masks_in[d])
        bq_t = [const.tile([P, 1], F32, tag=f"bq{m}", name=f"bq{m}") for m in range(4)]
        bk_t = [const.tile([P, 1], F32, tag=f"bk{m}", name=f"bk{m}") for m in range(4)]
        for m in range(4):
            nc.sync.dma_start(bq_t[m][:], bq_in[m])
            nc.sync.dma_start(bk_t[m][:], bk_in[m])
        bvb = const.tile([P, HCOLS], F32)
        nc.sync.dma_start(bvb[:], bvb_in[:])
        bpb = const.tile([P, C], F32)
        nc.sync.dma_start(bpb[:], bpb_in[:])
        ident = const.tile([P, P], F32)
        make_identity(nc, ident[:])

        def body():
            with ExitStack() as ctx:
                # ---- persistent SBUF for this iteration ----
                qkv_pool = ctx.enter_context(tc.tile_pool(name="qkv", bufs=1))
                qT = [qkv_pool.tile([P, T], F32R, tag=f"qT{m}", name=f"qT{m}") for m in range(4)]
                kT = [qkv_pool.tile([P, T], F32R, tag=f"kT{m}", name=f"kT{m}") for m in range(4)]
                vaug = [
                    qkv_pool.tile([P, HPC, HD + 1], BF16, tag=f"v{t}", name=f"v{t}")
                    for t in range(TOKTILES)
                ]

                # ---- phase 1: load xT, compute v then qT/kT ----
                with ExitStack() as qctx:
                    xt_pool = qctx.enter_context(tc.tile_pool(name="xt", bufs=1))
                    xT = [xt_pool.tile([P, T], F32R, tag=f"xT{k}", name=f"xT{k}") for k in range(KC)]
                    stage = qctx.enter_context(tc.tile_pool(name="stage", bufs=2))
                    for k in range(KC):
                        nc.gpsimd.dma_start(xT[k][:], xt_in[k * P : (k + 1) * P, :])

                    ps = qctx.enter_context(
                        tc.tile_pool(name="qkv_ps", bufs=4, space="PSUM")
                    )
                    # v: out [128 tok, 512 vcols]
                    wrv = stage.tile([P, KC, HCOLS], F32R, tag="wrv", bufs=1)
                    nc.gpsimd.dma_start(
                        wrv[:],
                        w3_in[:, 2 * HCOLS : 3 * HCOLS].rearrange(
                            "(kc p) m -> p kc m", p=P
                        ),
                    )
                    for t in range(TOKTILES):
                        pt = ps.tile([P, HCOLS], F32, tag="v_ps", bufs=4, name="pt")
                        for k in range(KC):
                            nc.tensor.matmul(
                                pt[:],
                                xT[k][:, t * P : (t + 1) * P],
                                wrv[:, k, :],
                                start=(k == 0),
                                stop=(k == KC - 1),
                            )
                        nc.vector.memset(vaug[t][:, :, HD : HD + 1], 1.0)
                        nc.vector.tensor_add(
                            vaug[t][:, :, 0:HD],
                            pt[:].rearrange("p (h d) -> p h d", d=HD),
                            bvb[:].rearrange("p (h d) -> p h d", d=HD),
                        )
                    # qT / kT: out [128 cols, 512 tok]
                    for m in range(4):
                        for part, dst, bias in ((0, qT, bq_t), (1, kT, bk_t)):
                            wr = stage.tile([P, KC, P], F32R, tag="wr")
                            col0 = part * HCOLS + m * P
                            nc.gpsimd.dma_start(
                                wr[:],
                                w3_in[:, col0 : col0 + P].rearrange(
                                    "(kc p) m -> p kc m", p=P
                                ),
                            )
                            for tck in range(4):
                                pt = ps.tile([P, 512], F32, tag="qk_ps", bufs=4, name="pt")
                                for k in range(KC):
                                    nc.tensor.matmul(
                                        pt[:],
                                        wr[:, k, :],
                                        xT[k][:, tck * 512 : (tck + 1) * 512],
                                        start=(k == 0),
                                        stop=(k == KC - 1),
                                    )
                                nc.vector.tensor_scalar_add(
                                    dst[m][:, tck * 512 : (tck + 1) * 512],
                                    pt[:],
                                    bias[m][:],
                                )

                # ---- phase 2: attention (qc outer) + transpose + proj ----
                y_pool = ctx.enter_context(tc.tile_pool(name="y", bufs=1))
                y = [y_pool.tile([P, HCOLS], F32, tag=f"y{t}", name=f"y{t}") for t in range(TOKTILES)]
                with ExitStack() as actx:
                    att_sb = actx.enter_context(tc.tile_pool(name="att_sb", bufs=6))
                    s_ps = actx.enter_context(
                        tc.tile_pool(name="s_ps", bufs=2, space="PSUM")
                    )
                    pv_ps = actx.enter_context(
                        tc.tile_pool(name="pv_ps", bufs=1, space="PSUM")
                    )
                    sm_pool = actx.enter_context(tc.tile_pool(name="sm", bufs=4))
                    yt_pool = actx.enter_context(tc.tile_pool(name="yt", bufs=1))
                    yT = [yt_pool.tile([P, T], F32R, tag=f"yT{m}", name=f"yT{m}") for m in range(4)]
                    proj_ps = actx.enter_context(
                        tc.tile_pool(name="proj_ps", bufs=1, space="PSUM")
                    )
                    stage2 = actx.enter_context(tc.tile_pool(name="stage2", bufs=3))
                    wpr = stage2.tile([P, 4, C], F32R, tag="wpr", bufs=1)
                    nc.gpsimd.dma_start(
                        wpr[:], wp_in[:].rearrange("(kc p) m -> p kc m", p=P)
                    )

                    for qc in range(4):
                        nk = 4 * qc + 4
                        for h in range(HPC):
                            m = h // 2
                            hsl = slice((h % 2) * HD, (h % 2) * HD + HD)
                            pv = [
                                pv_ps.tile([P, HD + 1], F32, tag=f"pv{s}", name=f"pv{s}")
                                for s in range(4)
                            ]
                            for kt in range(nk):
                                sp = s_ps.tile([P, 512], F32, tag="sp")
                                nc.tensor.matmul(
                                    sp[:],
                                    kT[m][hsl, kt * P : (kt + 1) * P],
                                    qT[m][hsl, qc * 512 : (qc + 1) * 512],
                                    start=True,
                                    stop=True,
                                )
                                at = att_sb.tile([P, 512], BF16, tag="at")
                                nc.scalar.activation(
                                    at[:],
                                    sp[:],
                                    mybir.ActivationFunctionType.Exp,
                                    scale=0.125,
                                )
                                d = kt * P - qc * 512
                                if d >= 0:
                                    nc.vector.tensor_mul(at[:], at[:], masks[d // P][:])
                                for s in range(4):
                                    nc.tensor.matmul(
                                        pv[s][:],
                                        at[:, s * P : (s + 1) * P],
                                        vaug[kt][:, h, :],
                                        start=(kt == 0),
                                        stop=(kt == nk - 1),
                                    )
                            for s in range(4):
                                t = qc * 4 + s
                                rec = sm_pool.tile([P, 1], F32, tag="rec")
                                nc.vector.reciprocal(rec[:], pv[s][:, HD : HD + 1])
                                nc.vector.tensor_scalar_mul(
                                    y[t][:, h * HD : (h + 1) * HD],
                                    pv[s][:, 0:HD],
                                    rec[:],
                                )
                        # transpose + proj for this qc's four token tiles
                        for s in range(4):
                            t = qc * 4 + s
                            for m in range(4):
                                tp = proj_ps.tile([P, P], F32, tag="tp", bufs=1, name="tp")
                                nc.tensor.transpose(
                                    tp[:], y[t][:, m * P : (m + 1) * P], ident[:]
                                )
                                nc.vector.tensor_copy(
                                    yT[m][:, t * P : (t + 1) * P], tp[:]
                                )
                            for ncol in range(2):
                                pt = proj_ps.tile([P, 512], F32, tag="pp", bufs=1, name="pt")
                                for k in range(4):
                                    nc.tensor.matmul(
                                        pt[:],
                                        yT[k][:, t * P : (t + 1) * P],
                                        wpr[:, k, ncol * 512 : (ncol + 1) * 512],
                                        start=(k == 0),
                                        stop=(k == 3),
                                    )
                                ob = stage2.tile([P, 512], F32, tag="ob")
                                nc.vector.tensor_add(
                                    ob[:], pt[:], bpb[:, ncol * 512 : (ncol + 1) * 512]
                                )
                                nc.sync.dma_start(
                                    partial[qc][
                                        s * P : (s + 1) * P,
                                        ncol * 512 : (ncol + 1) * 512,
                                    ],
                                    ob[:],
                                )
                        # pairwise reduce-scatter for this group (overlaps
                        # with the next group's attention)
                        nc.gpsimd.collective_compute(
                            "ReduceScatter",
                            mybir.AluOpType.add,
                            replica_groups=[[0, 1], [2, 3], [4, 5], [6, 7]],
                            ins=[partial[qc].opt()],
                            outs=[rs_out[qc].opt()],
                        )
                        nc.sync.dma_start(
                            out_p[qc * 256 : (qc + 1) * 256, :], rs_out[qc][:]
                        )


        for _ in range(reps):
            body()

    legalize_waits(nc)
    return nc


def prep_inputs(x, W_qkv, b_qkv, W_proj, b_proj):
    x = np.asarray(x, dtype=np.float32)
    W_qkv = np.asarray(W_qkv, dtype=np.float32)
    b_qkv = np.asarray(b_qkv, dtype=np.float32)
    W_proj = np.asarray(W_proj, dtype=np.float32)
    b_proj = np.asarray(b_proj, dtype=np.float32)

    xTs = [np.ascontiguousarray(x[b].T) for b in range(B)]
    halves = []
    for half in range(2):
        c0 = half * HCOLS
        w3 = np.ascontiguousarray(
            np.concatenate(
                [
                    W_qkv[:, c0 : c0 + HCOLS],
                    W_qkv[:, C + c0 : C + c0 + HCOLS],
                    W_qkv[:, 2 * C + c0 : 2 * C + c0 + HCOLS],
                ],
                axis=1,
            )
        )
        wp = np.ascontiguousarray(W_proj[c0 : c0 + HCOLS, :])
        bq = np.ascontiguousarray(b_qkv[c0 : c0 + HCOLS].reshape(4, P, 1))
        bk = np.ascontiguousarray(b_qkv[C + c0 : C + c0 + HCOLS].reshape(4, P, 1))
        bvb = np.tile(b_qkv[2 * C + c0 : 2 * C + c0 + HCOLS], (P, 1))
        halves.append((w3, wp, bq, bk, np.ascontiguousarray(bvb)))
    # both cores of a pair add the proj bias before the ReduceScatter
    # sums them, so each adds half
    bpb = np.ascontiguousarray(np.tile(b_proj / 2.0, (P, 1)))

    kk = np.arange(P)[:, None]
    qq = np.arange(512)[None, :]
    masks = np.stack(
        [(d * P + kk <= qq) for d in range(4)]
    ).astype(ml_dtypes.bfloat16)

    in_maps = []
    for c in range(NCORES):
        b, half = c // 2, c % 2
        w3, wp, bq, bk, bvb = halves[half]
        in_maps.append(
            {
                "xt": xTs[b],
                "w3": w3,
                "wp": wp,
                "bq": bq,
                "bk": bk,
                "bvb": bvb,
                "bpb": bpb,
                "masks": masks,
            }
        )
    return in_maps


class _Runner:
    """Build-once SPMD executor via PJRT (mirrors bass2jax.run_bass_via_pjrt)."""

    def __init__(self, nc, n_cores=NCORES):
        import jax
        from jax.sharding import Mesh, PartitionSpec, NamedSharding
        from jax.experimental.shard_map import shard_map
        from concourse.bass2jax import (
            _bass_exec_p,
            install_neuronx_cc_hook,
            partition_id_tensor,
        )

        self.jax = jax
        install_neuronx_cc_hook()
        partition_name = (
            nc.partition_id_tensor.name if nc.partition_id_tensor else None
        )
        in_names, out_names, out_avals, zero_outs = [], [], [], []
        for alloc in nc.m.functions[0].allocations:
            if not isinstance(alloc, mybir.MemoryLocationSet):
                continue
            name = alloc.memorylocations[0].name
            if alloc.kind == "ExternalInput":
                if name != partition_name:
                    in_names.append(name)
            elif alloc.kind == "ExternalOutput":
                shape = tuple(alloc.tensor_shape)
                dtype = mybir.dt.np(alloc.dtype)
                out_names.append(name)
                out_avals.append(jax.core.ShapedArray(shape, dtype))
                zero_outs.append(np.zeros(shape, dtype))
        self.in_names, self.out_names = in_names, out_names
        self.out_avals, self.zero_outs = out_avals, zero_outs
        self.n_cores = n_cores
        n_params = len(in_names)
        self.n_params = n_params
        all_in = list(in_names) + list(out_names)
        if partition_name is not None:
            all_in.append(partition_name)
        donate = tuple(range(n_params, n_params + len(out_names)))

        def _body(*args):
            operands = list(args)
            if partition_name is not None:
                operands.append(partition_id_tensor())
            outs = _bass_exec_p.bind(
                *operands,
                out_avals=tuple(out_avals),
                in_names=tuple(all_in),
                out_names=tuple(out_names),
                lowering_input_output_aliases=(),
                sim_require_finite=True,
                sim_require_nnan=True,
                nc=nc,
            )
            return tuple(outs)

        devices = jax.devices()[:n_cores]
        self.mesh = Mesh(np.asarray(devices), ("core",))
        in_specs = (PartitionSpec("core"),) * (n_params + len(out_names))
        out_specs = (PartitionSpec("core"),) * len(out_names)
        self.sharding = NamedSharding(self.mesh, PartitionSpec("core"))
        self.jitted = jax.jit(
            shard_map(
                _body,
                mesh=self.mesh,
                in_specs=in_specs,
                out_specs=out_specs,
                check_rep=False,
            ),
            donate_argnums=donate,
            keep_unused=True,
        )

    def put_inputs(self, in_maps):
        per_core = [[np.asarray(m[n]) for n in self.in_names] for m in in_maps]
        concat = [
            np.concatenate([per_core[c][i] for c in range(self.n_cores)], axis=0)
            for i in range(self.n_params)
        ]
        return [self.jax.device_put(a, self.sharding) for a in concat]

    def _zeros(self):
        return [
            self.jax.device_put(
                np.zeros((self.n_cores * z.shape[0], *z.shape[1:]), z.dtype),
                self.sharding,
            )
            for z in self.zero_outs
        ]

    def run(self, dev_inputs, n_timed=0):
        out = self.jitted(*dev_inputs, *self._zeros())
        self.jax.block_until_ready(out)
        times = []
        for _ in range(n_timed):
            z = self._zeros()
            self.jax.block_until_ready(z)
            t0 = time.perf_counter()
            out2 = self.jitted(*dev_inputs, *z)
            self.jax.block_until_ready(out2)
            times.append(time.perf_counter() - t0)
            out = out2
        np_outs = [np.asarray(a) for a in out]
        results = [
            {
                n: np_outs[i].reshape(self.n_cores, *self.out_avals[i].shape)[c]
                for i, n in enumerate(self.out_names)
            }
            for c in range(self.n_cores)
        ]
        return results, times


_RUNNERS = {}


def get_runner(reps: int = 1) -> _Runner:
    if reps not in _RUNNERS:
        _RUNNERS[reps] = _Runner(build_nc(reps))
    return _RUNNERS[reps]


def kernel(x, W_qkv, b_qkv, W_proj, b_proj):
    in_maps = prep_inputs(x, W_qkv, b_qkv, W_proj, b_proj)
    runner = get_runner(1)
    results, _ = runner.run(runner.put_inputs(in_maps))
    out = np.empty((B, T, C), dtype=np.float32)
    for c in range(NCORES):
        b, rank = c // 2, c % 2
        part = results[c]["out_part"]
        for g in range(4):
            r0 = g * 512 + rank * 256
            out[b, r0 : r0 + 256, :] = part[g * 256 : (g + 1) * 256, :]
    return out
